# revision 12
# baseline (speedup 1.0000x reference)
"""Causal linear attention (elu+1 feature map, diagonal kv cumsum) on 8 TRN2 cores.

Math per (b, h) slice (L=4096 positions, D=64):
    qf = elu(q) + 1 = exp(-relu(-q)) + relu(q)        (exact identity)
    kf = elu(k) + 1
    kcum  = cumsum_L(kf)
    kvcum = cumsum_L(kf * v)
    z   = sum_D(qf * kcum) + eps
    out = qf * kvcum / z

Sharding: (B*H)=64 slices -> 8 per core (pure data parallel, no collectives).

Layout on core: chunks of 128 consecutive positions on the 128 SBUF
partitions, D=64 on the free dim. Groups of 8 chunks = [128, 512] tiles.
Cumsum = per-chunk triangular matmul on TensorE + two-level prefix:
chunk totals (PSUM row 127) -> SBUF row -> SBUF-to-SBUF DMA redistribute to
[8, 64] -> strict-triangular prefix matmul -> K=1 broadcast matmul adds the
per-chunk prefix to every partition of the chunk's PSUM block.
"""

import sys

import numpy as np

for _p in ("/opt/trn_rl_repo", "/root/.axon_site/_ro/trn_rl_repo"):
    if _p not in sys.path:
        sys.path.append(_p)

import concourse.bass as bass
import concourse.mybir as mybir
import concourse.tile as tile
from concourse import bacc
from concourse.bass_utils import run_bass_kernel_spmd

B, H, L, D = 4, 16, 4096, 64
NCORES = 8
S = (B * H) // NCORES  # slices per core = 8
CH = 128               # positions per chunk (tri-matmul size)
NCG = 8                # chunks per group
GPOS = CH * NCG        # positions per group = 1024
NG = L // GPOS         # groups per slice = 4
EPS = 1e-6
FP = mybir.dt.float32

# float32r runs the big N=512 matmuls at ~4x the fp32 rate; flip off if
# accuracy degrades.
USE_F32R_TRI = False
USE_F32R_BCAST = False

AF = mybir.ActivationFunctionType
OP = mybir.AluOpType


def _mmdt(ap, use_f32r):
    return ap.bitcast(mybir.dt.float32r) if use_f32r else ap


def build_nc():
    nc = bacc.Bacc()
    q = nc.declare_dram_parameter("q", [S, L, D], FP, isOutput=False)
    k = nc.declare_dram_parameter("k", [S, L, D], FP, isOutput=False)
    v = nc.declare_dram_parameter("v", [S, L, D], FP, isOutput=False)
    cti = nc.declare_dram_parameter("cti", [CH, CH], FP, isOutput=False)
    cones128 = nc.declare_dram_parameter("cones128", [1, CH], FP, isOutput=False)
    ctstrict = nc.declare_dram_parameter("ctstrict", [NCG, NCG + 1], FP, isOutput=False)
    cones9 = nc.declare_dram_parameter("cones9", [1, NCG + 1], FP, isOutput=False)
    out = nc.declare_dram_parameter("out", [S, L, D], FP, isOutput=True)

    # position l = g*1024 + c*128 + p  ->  [s, g, p, c, d]
    q_r = q.rearrange("s (g c p) d -> s g p c d", c=NCG, p=CH)
    k_r = k.rearrange("s (g c p) d -> s g p c d", c=NCG, p=CH)
    v_r = v.rearrange("s (g c p) d -> s g p c d", c=NCG, p=CH)
    out_r = out.rearrange("s (g c p) d -> s g p c d", c=NCG, p=CH)

    with tile.TileContext(nc) as tc:
        with (
            tc.tile_pool(name="singles", bufs=1) as singles,
            tc.tile_pool(name="io", bufs=3) as iop,
            tc.tile_pool(name="work", bufs=2) as wp,
            tc.tile_pool(name="small", bufs=4) as sp,
            tc.tile_pool(name="psum", bufs=2, space="PSUM") as pp,
            tc.tile_pool(name="psum_pref", bufs=2, space="PSUM") as pps,
            tc.tile_pool(name="dram", bufs=4, space="DRAM") as dp,
        ):
            ti_sb = singles.tile([CH, CH], FP)
            nc.sync.dma_start(out=ti_sb, in_=cti[:])
            ones128_sb = singles.tile([1, CH], FP)
            nc.sync.dma_start(out=ones128_sb, in_=cones128[:])
            tstrict_sb = singles.tile([NCG, NCG + 1], FP)
            nc.sync.dma_start(out=tstrict_sb, in_=ctstrict[:])
            ones9_sb = singles.tile([1, NCG + 1], FP)
            nc.sync.dma_start(out=ones9_sb, in_=cones9[:])

            for s in range(S):
                flat_prev = None
                for g in range(NG):
                    # ---- load q, k, v ----
                    q_t = iop.tile([CH, NCG, D], FP, tag="q")
                    k_t = iop.tile([CH, NCG, D], FP, tag="k")
                    v_t = iop.tile([CH, NCG, D], FP, tag="v")
                    nc.sync.dma_start(out=q_t, in_=q_r[s, g])
                    nc.sync.dma_start(out=k_t, in_=k_r[s, g])
                    nc.sync.dma_start(out=v_t, in_=v_r[s, g])

                    # ---- em = exp(min(x, 0)) = exp(-relu(-x)) on ACT,
                    #      qf/kf = relu(x) + em  (= elu(x)+1) on DVE ----
                    emq_t = wp.tile([CH, NCG, D], FP, tag="emq")
                    nc.scalar.activation(out=emq_t, in_=q_t, func=AF.Relu, scale=-1.0)
                    nc.scalar.activation(out=emq_t, in_=emq_t, func=AF.Exp, scale=-1.0)
                    emk_t = wp.tile([CH, NCG, D], FP, tag="emk")
                    nc.scalar.activation(out=emk_t, in_=k_t, func=AF.Relu, scale=-1.0)
                    nc.scalar.activation(out=emk_t, in_=emk_t, func=AF.Exp, scale=-1.0)

                    qfkf_t = wp.tile([CH, 2, NCG, D], FP, tag="qfkf")
                    nc.vector.scalar_tensor_tensor(
                        out=qfkf_t[:, 0], in0=q_t, scalar=0.0, in1=emq_t,
                        op0=OP.max, op1=OP.add,
                    )
                    nc.vector.scalar_tensor_tensor(
                        out=qfkf_t[:, 1], in0=k_t, scalar=0.0, in1=emk_t,
                        op0=OP.max, op1=OP.add,
                    )
                    kv_t = wp.tile([CH, NCG, D], FP, tag="kv")
                    nc.vector.tensor_mul(kv_t, qfkf_t[:, 1], v_t)

                    # ---- within-chunk cumsums on PE (PSUM [128, 1024]) ----
                    ps_t = pp.tile([CH, 2, NCG, D], FP, tag="ps")
                    nc.tensor.matmul(
                        out=ps_t[:, 0],
                        lhsT=_mmdt(ti_sb[:], USE_F32R_TRI),
                        rhs=_mmdt(qfkf_t[:, 1], USE_F32R_TRI),
                        start=True, stop=False,
                    )
                    nc.tensor.matmul(
                        out=ps_t[:, 1],
                        lhsT=_mmdt(ti_sb[:], USE_F32R_TRI),
                        rhs=_mmdt(kv_t[:], USE_F32R_TRI),
                        start=True, stop=False,
                    )

                    # ---- chunk totals: PSUM row 127 -> SBUF row -> [8,2,64] ----
                    # totrow layout (c, s, d) so the redistribute DMA merges
                    # (s, d) into one contiguous 128-elem run per chunk.
                    # (engine partition base must be 32-aligned: copy 96:128,
                    # only row 127 is used downstream)
                    totrow_t = wp.tile([CH, NCG, 2, D], FP, tag="totrow")
                    nc.scalar.copy(
                        out=totrow_t[96:128],
                        in_=ps_t[96:128].rearrange("o s c d -> o c s d"),
                    )
                    tot_b = dp.tile([NCG, 2 * D], FP, tag="tot_b")
                    nc.sync.dma_start(
                        out=tot_b[:].unsqueeze(0),
                        in_=totrow_t[127:128].rearrange("o c s d -> o c (s d)"),
                    )
                    tot_t = sp.tile([NCG, 2, D], FP, tag="tot")
                    nc.sync.dma_start(
                        out=tot_t.rearrange("c s d -> c (s d)"), in_=tot_b[:],
                    )

                    # ---- prefix matmul: rows 0..7 excl-prefix, row 8 carry.
                    # Both K|V halves in ONE matmul: interleaving two start=True
                    # accumulation groups in the same PSUM bank wipes the first
                    # group's has_written state (K side lost its tstrict part).
                    pref_ps = pps.tile([NCG + 1, 2, D], FP, tag="pref")
                    nc.tensor.matmul(
                        out=pref_ps, lhsT=tstrict_sb[:], rhs=tot_t[:],
                        start=True, stop=(g == 0),
                    )
                    if g > 0:
                        nc.tensor.matmul(
                            out=pref_ps, lhsT=ones9_sb[:],
                            rhs=flat_prev[0:1, NCG, :, :],
                            start=False, stop=True,
                        )

                    prefsb_t = sp.tile([NCG + 1, 2, D], FP, tag="prefsb")
                    nc.scalar.copy(out=prefsb_t, in_=pref_ps)
                    # flatten the 9 partition rows into one [1, 9, 2, 64] row
                    # via a DRAM bounce (direct N->1 partition SBUF-to-SBUF
                    # DMA produces garbage on TRN2)
                    bounce_t = dp.tile([NCG + 1, 2, D], FP, tag="bounce")
                    nc.sync.dma_start(out=bounce_t, in_=prefsb_t[:])
                    flat_t = sp.tile([1, NCG + 1, 2, D], FP, tag="flat")
                    nc.sync.dma_start(
                        out=flat_t[:].rearrange("o c s d -> o (c s d)"),
                        in_=bounce_t[:].rearrange("c s d -> (c s d)").unsqueeze(0),
                    )

                    # ---- broadcast-add prefixes into PSUM (K=1 matmul) ----
                    nc.tensor.matmul(
                        out=ps_t[:, 0],
                        lhsT=_mmdt(ones128_sb[:], USE_F32R_BCAST),
                        rhs=_mmdt(flat_t[0:1, 0:NCG, 0, :], USE_F32R_BCAST),
                        start=False, stop=False,
                    )
                    nc.tensor.matmul(
                        out=ps_t[:, 1],
                        lhsT=_mmdt(ones128_sb[:], USE_F32R_BCAST),
                        rhs=_mmdt(flat_t[0:1, 0:NCG, 1, :], USE_F32R_BCAST),
                        start=False, stop=True,
                    )
                    flat_prev = flat_t

                    # ---- epilogue: qk|num, z, reciprocal, out ----
                    qknum_t = wp.tile([CH, 2, NCG, D], FP, tag="qknum")
                    qf_b = qfkf_t[:, 0:1].broadcast_to([CH, 2, NCG, D])
                    nc.vector.tensor_mul(qknum_t, qf_b, ps_t)

                    z_t = sp.tile([CH, NCG], FP, tag="z")
                    nc.vector.tensor_reduce(
                        out=z_t, in_=qknum_t[:, 0], axis=mybir.AxisListType.X,
                        op=OP.add,
                    )
                    rz_t = sp.tile([CH, NCG], FP, tag="rz")
                    nc.vector.tensor_scalar_add(rz_t, z_t, EPS)
                    nc.vector.reciprocal(out=rz_t, in_=rz_t)

                    o_t = iop.tile([CH, NCG, D], FP, tag="o")
                    rz_b = rz_t[:].unsqueeze(2).broadcast_to([CH, NCG, D])
                    nc.vector.tensor_mul(o_t, qknum_t[:, 1], rz_b)
                    nc.sync.dma_start(out=out_r[s, g], in_=o_t)

    nc.compile()
    return nc


def consts():
    return {
        "cti": np.triu(np.ones((CH, CH), np.float32)),
        "cones128": np.ones((1, CH), np.float32),
        "ctstrict": np.triu(np.ones((NCG, NCG + 1), np.float32), k=1),
        "cones9": np.ones((1, NCG + 1), np.float32),
    }


_NC_CACHE = None


def _get_nc():
    global _NC_CACHE
    if _NC_CACHE is None:
        _NC_CACHE = build_nc()
    return _NC_CACHE


def run(q, k, v, trace=False):
    nc = _get_nc()
    cs = consts()
    q3 = np.ascontiguousarray(q.reshape(B * H, L, D).astype(np.float32, copy=False))
    k3 = np.ascontiguousarray(k.reshape(B * H, L, D).astype(np.float32, copy=False))
    v3 = np.ascontiguousarray(v.reshape(B * H, L, D).astype(np.float32, copy=False))
    in_maps = []
    for i in range(NCORES):
        sl = slice(i * S, (i + 1) * S)
        in_maps.append({"q": q3[sl], "k": k3[sl], "v": v3[sl], **cs})
    res = run_bass_kernel_spmd(
        nc, in_maps, core_ids=list(range(NCORES)), trace=trace,
    )
    outs = [res.results[i]["out"] for i in range(NCORES)]
    full = np.concatenate(outs, axis=0).reshape(B, H, L, D)
    return full, res


def kernel(q, k, v, attention_mask=None):
    # attention_mask is all-ones for this problem shape (fill=ones); the
    # masked reference reduces to the unmasked computation.
    full, _ = run(np.asarray(q), np.asarray(k), np.asarray(v))
    return full


# revision 16
# speedup vs baseline: 1.3264x; 1.3264x over previous
"""Causal linear attention (elu+1 feature map, diagonal kv cumsum) on 8 TRN2 cores.

Math per (b, h) slice (L=4096 positions, D=64):
    qf = elu(q) + 1 = exp(-relu(-q)) + relu(q)        (exact identity)
    kf = elu(k) + 1
    kcum  = cumsum_L(kf)
    kvcum = cumsum_L(kf * v)
    z   = sum_D(qf * kcum) + eps
    out = qf * kvcum / z

Sharding: (B*H)=64 slices -> 8 per core (pure data parallel, no collectives).

Layout on core: chunks of 128 consecutive positions on the 128 SBUF
partitions, D=64 on the free dim. Groups of 8 chunks = [128, 512] tiles.
Cumsum = per-chunk triangular matmul on TensorE + two-level prefix:
chunk totals (PSUM row 127) -> SBUF row -> SBUF-to-SBUF DMA redistribute to
[8, 64] -> strict-triangular prefix matmul -> K=1 broadcast matmul adds the
per-chunk prefix to every partition of the chunk's PSUM block.
"""

import sys

import numpy as np

for _p in ("/opt/trn_rl_repo", "/root/.axon_site/_ro/trn_rl_repo"):
    if _p not in sys.path:
        sys.path.append(_p)

import concourse.bass as bass
import concourse.mybir as mybir
import concourse.tile as tile
from concourse import bacc
from concourse.bass_utils import run_bass_kernel_spmd

B, H, L, D = 4, 16, 4096, 64
NCORES = 8
S = (B * H) // NCORES  # slices per core = 8
CH = 128               # positions per chunk (tri-matmul size)
NCG = 8                # chunks per group
GPOS = CH * NCG        # positions per group = 1024
NG = L // GPOS         # groups per slice = 4
EPS = 1e-6
FP = mybir.dt.float32

# float32r runs the big N=512 matmuls at ~4x the fp32 rate; flip off if
# accuracy degrades.
USE_F32R_TRI = False
USE_F32R_BCAST = False

BF = mybir.dt.bfloat16
USE_BF16_MM = True

AF = mybir.ActivationFunctionType
OP = mybir.AluOpType


def _mmdt(ap, use_f32r):
    return ap.bitcast(mybir.dt.float32r) if use_f32r else ap


def build_nc():
    nc = bacc.Bacc()
    q = nc.declare_dram_parameter("q", [S, L, D], FP, isOutput=False)
    k = nc.declare_dram_parameter("k", [S, L, D], FP, isOutput=False)
    v = nc.declare_dram_parameter("v", [S, L, D], FP, isOutput=False)
    CDT = BF if USE_BF16_MM else FP
    cti = nc.declare_dram_parameter("cti", [CH, CH], CDT, isOutput=False)
    cones128 = nc.declare_dram_parameter("cones128", [1, CH], CDT, isOutput=False)
    ctstrict = nc.declare_dram_parameter("ctstrict", [NCG, NCG + 1], CDT, isOutput=False)
    cones9 = nc.declare_dram_parameter("cones9", [1, NCG + 1], CDT, isOutput=False)
    out = nc.declare_dram_parameter("out", [S, L, D], FP, isOutput=True)

    # position l = g*1024 + c*128 + p  ->  [s, g, p, c, d]
    q_r = q.rearrange("s (g c p) d -> s g p c d", c=NCG, p=CH)
    k_r = k.rearrange("s (g c p) d -> s g p c d", c=NCG, p=CH)
    v_r = v.rearrange("s (g c p) d -> s g p c d", c=NCG, p=CH)
    out_r = out.rearrange("s (g c p) d -> s g p c d", c=NCG, p=CH)

    with tile.TileContext(nc) as tc:
        with (
            tc.tile_pool(name="singles", bufs=1) as singles,
            tc.tile_pool(name="io", bufs=3) as iop,
            tc.tile_pool(name="work", bufs=2) as wp,
            tc.tile_pool(name="small", bufs=4) as sp,
            tc.tile_pool(name="psum", bufs=2, space="PSUM") as pp,
            tc.tile_pool(name="psum_pref", bufs=2, space="PSUM") as pps,
            tc.tile_pool(name="dram", bufs=4, space="DRAM") as dp,
        ):
            MDT = BF if USE_BF16_MM else FP
            ti_sb = singles.tile([CH, CH], MDT)
            nc.sync.dma_start(out=ti_sb, in_=cti[:])
            ones128_sb = singles.tile([1, CH], MDT)
            nc.sync.dma_start(out=ones128_sb, in_=cones128[:])
            tstrict_sb = singles.tile([NCG, NCG + 1], MDT)
            nc.sync.dma_start(out=tstrict_sb, in_=ctstrict[:])
            ones9_sb = singles.tile([1, NCG + 1], MDT)
            nc.sync.dma_start(out=ones9_sb, in_=cones9[:])

            for s in range(S):
                flat_prev = None
                for g in range(NG):
                    # ---- load q, k, v ----
                    q_t = iop.tile([CH, NCG, D], FP, tag="q")
                    k_t = iop.tile([CH, NCG, D], FP, tag="k")
                    v_t = iop.tile([CH, NCG, D], FP, tag="v")
                    nc.sync.dma_start(out=q_t, in_=q_r[s, g])
                    nc.sync.dma_start(out=k_t, in_=k_r[s, g])
                    nc.sync.dma_start(out=v_t, in_=v_r[s, g])

                    # ---- em = exp(min(x, 0)) = exp(-relu(-x)) on ACT,
                    #      qf/kf = relu(x) + em  (= elu(x)+1) on DVE ----
                    emq_t = wp.tile([CH, NCG, D], FP, tag="emq")
                    nc.scalar.activation(out=emq_t, in_=q_t, func=AF.Relu, scale=-1.0)
                    nc.scalar.activation(out=emq_t, in_=emq_t, func=AF.Exp, scale=-1.0)
                    emk_t = wp.tile([CH, NCG, D], FP, tag="emk")
                    nc.scalar.activation(out=emk_t, in_=k_t, func=AF.Relu, scale=-1.0)
                    nc.scalar.activation(out=emk_t, in_=emk_t, func=AF.Exp, scale=-1.0)

                    qfkf_t = wp.tile([CH, 2, NCG, D], MDT, tag="qfkf")
                    nc.vector.scalar_tensor_tensor(
                        out=qfkf_t[:, 0], in0=q_t,
                        scalar=0.0, in1=emq_t, op0=OP.max, op1=OP.add,
                    )
                    nc.vector.scalar_tensor_tensor(
                        out=qfkf_t[:, 1], in0=k_t,
                        scalar=0.0, in1=emk_t, op0=OP.max, op1=OP.add,
                    )
                    kv_t = wp.tile([CH, NCG, D], MDT, tag="kv")
                    nc.vector.tensor_tensor(
                        out=kv_t[:], in0=qfkf_t[:, 1],
                        in1=v_t[:], op=OP.mult,
                    )

                    # ---- within-chunk cumsums on PE (PSUM [128, 1024]) ----
                    ps_t = pp.tile([CH, 2, NCG, D], FP, tag="ps")
                    nc.tensor.matmul(
                        out=ps_t[:, 0],
                        lhsT=ti_sb[:],
                        rhs=qfkf_t[:, 1],
                        start=True, stop=False,
                    )
                    nc.tensor.matmul(
                        out=ps_t[:, 1],
                        lhsT=ti_sb[:],
                        rhs=kv_t[:],
                        start=True, stop=False,
                    )

                    # ---- chunk totals: PSUM row 127 -> SBUF row -> [8,2,64] ----
                    # totrow layout (c, s, d) so the redistribute DMA merges
                    # (s, d) into one contiguous 128-elem run per chunk.
                    # (engine partition base must be 32-aligned: copy 96:128,
                    # only row 127 is used downstream)
                    totrow_t = wp.tile([CH, NCG, 2, D], MDT, tag="totrow")
                    nc.scalar.copy(
                        out=totrow_t[96:128],
                        in_=ps_t[96:128].rearrange("o s c d -> o c s d"),
                    )
                    tot_b = dp.tile([NCG, 2 * D], MDT, tag="tot_b")
                    nc.sync.dma_start(
                        out=tot_b[:].unsqueeze(0),
                        in_=totrow_t[127:128].rearrange("o c s d -> o c (s d)"),
                    )
                    tot_t = sp.tile([NCG, 2, D], MDT, tag="tot")
                    nc.sync.dma_start(
                        out=tot_t.rearrange("c s d -> c (s d)"), in_=tot_b[:],
                    )

                    # ---- prefix matmul: rows 0..7 excl-prefix, row 8 carry.
                    # Both K|V halves in ONE matmul: interleaving two start=True
                    # accumulation groups in the same PSUM bank wipes the first
                    # group's has_written state (K side lost its tstrict part).
                    pref_ps = pps.tile([NCG + 1, 2, D], FP, tag="pref")
                    nc.tensor.matmul(
                        out=pref_ps, lhsT=tstrict_sb[:], rhs=tot_t[:],
                        start=True, stop=(g == 0),
                    )
                    if g > 0:
                        nc.tensor.matmul(
                            out=pref_ps, lhsT=ones9_sb[:],
                            rhs=flat_prev[0:1, NCG, :, :],
                            start=False, stop=True,
                        )

                    prefsb_t = sp.tile([NCG + 1, 2, D], MDT, tag="prefsb")
                    nc.scalar.copy(out=prefsb_t, in_=pref_ps)
                    # flatten the 9 partition rows into one [1, 9, 2, 64] row
                    # via a DRAM bounce (direct N->1 partition SBUF-to-SBUF
                    # DMA produces garbage on TRN2)
                    bounce_t = dp.tile([NCG + 1, 2, D], MDT, tag="bounce")
                    nc.sync.dma_start(out=bounce_t, in_=prefsb_t[:])
                    flat_t = sp.tile([1, NCG + 1, 2, D], MDT, tag="flat")
                    nc.sync.dma_start(
                        out=flat_t[:].rearrange("o c s d -> o (c s d)"),
                        in_=bounce_t[:].rearrange("c s d -> (c s d)").unsqueeze(0),
                    )

                    # ---- broadcast-add prefixes into PSUM (K=1 matmul) ----
                    nc.tensor.matmul(
                        out=ps_t[:, 0],
                        lhsT=ones128_sb[:],
                        rhs=flat_t[0:1, 0:NCG, 0, :],
                        start=False, stop=False,
                    )
                    nc.tensor.matmul(
                        out=ps_t[:, 1],
                        lhsT=ones128_sb[:],
                        rhs=flat_t[0:1, 0:NCG, 1, :],
                        start=False, stop=True,
                    )
                    flat_prev = flat_t

                    # ---- epilogue: qk|num, z, reciprocal, out ----
                    qknum_t = wp.tile([CH, 2, NCG, D], FP, tag="qknum")
                    qf_b = qfkf_t[:, 0:1].broadcast_to([CH, 2, NCG, D])
                    nc.vector.tensor_mul(qknum_t, qf_b, ps_t)

                    z_t = sp.tile([CH, NCG], FP, tag="z")
                    nc.vector.tensor_reduce(
                        out=z_t, in_=qknum_t[:, 0], axis=mybir.AxisListType.X,
                        op=OP.add,
                    )
                    rz_t = sp.tile([CH, NCG], FP, tag="rz")
                    nc.vector.tensor_scalar_add(rz_t, z_t, EPS)
                    nc.vector.reciprocal(out=rz_t, in_=rz_t)

                    o_t = iop.tile([CH, NCG, D], FP, tag="o")
                    rz_b = rz_t[:].unsqueeze(2).broadcast_to([CH, NCG, D])
                    nc.vector.tensor_mul(o_t, qknum_t[:, 1], rz_b)
                    nc.sync.dma_start(out=out_r[s, g], in_=o_t)

    nc.compile()
    return nc


def consts():
    import ml_dtypes

    cdt = ml_dtypes.bfloat16 if USE_BF16_MM else np.float32
    return {
        "cti": np.triu(np.ones((CH, CH), cdt)),
        "cones128": np.ones((1, CH), cdt),
        "ctstrict": np.triu(np.ones((NCG, NCG + 1), cdt), k=1),
        "cones9": np.ones((1, NCG + 1), cdt),
    }


_NC_CACHE = None


def _get_nc():
    global _NC_CACHE
    if _NC_CACHE is None:
        _NC_CACHE = build_nc()
    return _NC_CACHE


def run(q, k, v, trace=False):
    nc = _get_nc()
    cs = consts()
    q3 = np.ascontiguousarray(q.reshape(B * H, L, D).astype(np.float32, copy=False))
    k3 = np.ascontiguousarray(k.reshape(B * H, L, D).astype(np.float32, copy=False))
    v3 = np.ascontiguousarray(v.reshape(B * H, L, D).astype(np.float32, copy=False))
    in_maps = []
    for i in range(NCORES):
        sl = slice(i * S, (i + 1) * S)
        in_maps.append({"q": q3[sl], "k": k3[sl], "v": v3[sl], **cs})
    res = run_bass_kernel_spmd(
        nc, in_maps, core_ids=list(range(NCORES)), trace=trace,
    )
    outs = [res.results[i]["out"] for i in range(NCORES)]
    full = np.concatenate(outs, axis=0).reshape(B, H, L, D)
    return full, res


def kernel(q, k, v, attention_mask=None):
    # attention_mask is all-ones for this problem shape (fill=ones); the
    # masked reference reduces to the unmasked computation.
    full, _ = run(np.asarray(q), np.asarray(k), np.asarray(v))
    return full
